# revision 54
# baseline (speedup 1.0000x reference)
"""DenseEnergyLoss Trainium2 kernel (ROI-gathered).

loss = WEIGHT * (-1/n) * sum_{k,i,j} A'[k,i] * G[i,j] * B'[k,j]

where (per image):
  f[i]  = [x/50, y/50, r/15, g/15, b/15]          (5-dim feature per pixel)
  G[i,j] = exp(f_i . f_j)                          (symmetric)
  e[i]  = exp(-0.5 |f_i|^2)
  B'[k,i] = seg_r[k,i] * e[i]
  A'[k,i] = seg_r[k,i] * gate[i] * e[i]
so that A' G B' == seg_r * gate * kern * seg_r with kern the bilateral kernel.

KEY REDUCTION: seg_r = segs * roi with roi in {0,1}, so pixels with roi==0
have B' == A' == 0 exactly and contribute nothing to the quadratic form.
Only the ~2032 active pixels per image matter -> gather them, pad to P=2048.

Sharding: 2 cores per image. G is processed in [128 x 512] tiles; symmetry
halves the tile count: for column band b only row blocks pb < 4*(b+1) are
computed. Each G tile feeds one accumulating matmul whose stationary packs
[B'^T | A'^T] (42 cols): the B' half covers the lower-left triangle term,
the A' half the transposed upper-right term (valid for s < 2b; uniform
across cores thanks to the parity split: core half h owns blocks 2s+h).

v1 DMA diet (vs the 27972ns baseline): the startup and the first half of
the main loop were DMA-starved (both DGE queue sets sustain only ~55
packets/us; baseline shipped ~1100 packets / 956KB). Changes:
  - stat: per-row-group column packing; each rg loads only its own slots'
    [15,128] chunks (one [15,256] DMA per rg) -> 30KB / 60 packets
    (was 120KB / 240).
  - abrep: only rows 0-41 ([A';B']) are read by the reduction; ship 84
    rows (two copies, placed at partitions 0:42 and 64:106) -> 344KB
    (was 512KB), as two 42-descriptor DMAs issued LAST on their queues
    so the 4KB packets never sit ahead of mov columns on the shared
    DMA engines.
  - bapt: one [128,336] DMA (672B/partition) instead of 3 column chunks.
  - no PSUM memsets: each band's first mm2 pair uses start=True; the
    band reduce is ONE STT over partitions 0:106 (rows 42:63 multiply
    garbage PSUM x garbage SBUF — per-partition isolated; the host masks
    those rows, plus 21:42/85:106 for band 0 which never writes the
    A-side rows). Output DMA ships only rows 0:106.
  - queue assignment: pair-0 inputs split across GpSimd (SWDGE,
    ~1.4us issue-to-land) and Sync (HWDGE, ~1.8-2.8us) so desc-gens run
    in parallel; pair-1 inputs on Sync+Scalar land one cadence later;
    band-3 pairs run evens-first ([0,2,1,3]) because pairs 0/2 share row
    groups 0/1 whose data lands first.

Device pipeline per tile pair: PE matmul (c=15 bf16 hi/lo-compensated
feature contraction, row-group packed x2) -> ScalarE exp ([128,1024]
PSUM->SBUF bf16) -> PE matmul x2 (col-strip packed at cols 0/64, bf16)
accumulating into a per-band PSUM bank -> DVE multiply+reduce per band.
Host sums the per-core [106, NB] partials (valid rows only).

Measured floors (this machine): ACT exp ~0.97ns/col streamed (10 pair
activations = ~10us, the loop floor); PE pair step mm2+mm1 ~1068-1087ns
(mm1/mm2 stationaries overlap in the PE array, so they serialize; 4-way
mm1 batching needs 4 G PSUM buffers which don't fit alongside the two
band accumulators). Startup floor ~3.2us: queue-start ~7.2 + desc-gen
~0.7 + issue-to-land ~1.4-1.8 + first mm1 pair. Output DMA + final
barrier ~5.4us fixed (HWDGE land latency ~2.4us dominates; an on-device
ones-matmul reduce to 2 packets measured NO faster — latency, not
packet count).

Rejected experiments, all measured on HW: GpSimd pow-as-exp (~170us/tile),
fp8 matmul (runtime failure), 5-row hi-only features (whole-core clock
drops ~19%), BAND=256 tiles (per-instruction fixed cost dominates),
grouped-partition DMA writes (framework write-region tracking mis-models
them), band-first exp splits, scalar-free startup + depth-2 lookahead
(Tile scheduler keeps mm1/mm2 alternating regardless), XBAR-transpose
bapt (0-stride dst pattern defeats write tracking -> false serialization
of every queue), 3-tiles-per-activation 1536 buffers (ACT is ~linear
per col; PE quadrant collisions worsen), split first-pair activations
(counting-semaphore waits cover the whole pair anyway), diagonal-
supertile triangle tiling (saves 1536 exp cols/core but the extra
per-instruction overheads on ACT/PE cost more).
"""

import numpy as np
import ml_dtypes

WEIGHT = 1e-07
SIGMA_RGB = 15.0
SIGMA_XY_EFF = 50.0  # SIGMA_XY * SCALE
IGNORE_LABEL = 255

N_IMG = 4
K_CLS = 21
H_DS = 64
P_FULL = H_DS * H_DS  # 4096 downsampled pixels
BAND = 512
BLK = 128
W2 = 2 * K_CLS  # 42: combined [B'|A'] stationary width

BF16 = ml_dtypes.bfloat16

_CACHE = {}


def _rg(s):
    # row-group for mm1 packing: pairs alternate {0,1} / {2,3}
    return 2 * ((s // 2) % 2) + (s % 2)


def _rg_slots(n_lslot):
    """slots owned by each row group, in ascending order."""
    return [[s for s in range(n_lslot) if _rg(s) == r] for r in range(4)]


def _build_program(n_lslot):
    """n_lslot: local row-block slots per core. P_act = 256 * n_lslot,
    NB = n_lslot // 2 bands of 512. n_lslot must be even."""
    import concourse.bacc as bacc
    import concourse.tile as tile
    from concourse import mybir

    f32 = mybir.dt.float32
    bf16 = mybir.dt.bfloat16

    nb = n_lslot // 2
    p_act = 256 * n_lslot
    rg_slots = _rg_slots(n_lslot)
    n_chunk = max(len(sl) for sl in rg_slots)
    # stat_src column offset of each rg's packed chunk run
    rg_coff = [128 * sum(len(rg_slots[q]) for q in range(r)) for r in range(4)]

    nc = bacc.Bacc("TRN2", target_bir_lowering=False, debug=False)

    # mov_src holds the 15 feature rows replicated twice (30 rows) so the
    # four per-rg DMAs spread their HBM reads over distinct DRAM rows.
    mov_d = nc.dram_tensor("mov_src", [30, p_act], bf16, kind="ExternalInput")
    stat_d = nc.dram_tensor(
        "stat_src", [15, n_lslot * BLK], bf16, kind="ExternalInput"
    )
    bapt_d = nc.dram_tensor("bapt", [128, n_lslot * W2], bf16, kind="ExternalInput")
    abrep_d = nc.dram_tensor("abrep_src", [42, p_act], bf16, kind="ExternalInput")
    acc_d = nc.dram_tensor("acc_out", [106, nb], f32, kind="ExternalOutput")

    with tile.TileContext(nc) as tc:
        with (
            tc.tile_pool(name="const", bufs=1) as cpool,
            tc.tile_pool(name="gpsum", bufs=3, space="PSUM") as gpool,
            tc.tile_pool(name="accpsum", bufs=2, space="PSUM") as apool,
            tc.tile_pool(name="gsb", bufs=3) as gsbpool,
            tc.tile_pool(name="scr", bufs=2) as scrpool,
        ):
            ft_stat = cpool.tile([128, 128 * n_chunk], bf16, tag="ftstat")
            ft_mov = cpool.tile([128, p_act], bf16, tag="ftmov")
            bapt = cpool.tile([128, n_lslot * W2], bf16, tag="bapt")
            abrep = cpool.tile([128, p_act], bf16, tag="abrep")
            acc = cpool.tile([128, nb], f32, tag="acc")

            # --- input DMAs, critical-first across the three DGE queues ---
            def stat_rg(r):
                w = 128 * len(rg_slots[r])
                return (
                    ft_stat[32 * r : 32 * r + 15, 0:w],
                    stat_d[0:15, rg_coff[r] : rg_coff[r] + w],
                )

            def mov_rg(rg, c0, c1):
                src_r = 15 * (rg % 2)
                return (
                    ft_mov[32 * rg : 32 * rg + 15, c0:c1],
                    mov_d[src_r : src_r + 15, c0:c1],
                )

            bands_desc = list(reversed(range(nb)))
            b_last = bands_desc[0]
            lb0, lb1 = b_last * BAND, (b_last + 1) * BAND
            rest_hi = lb0  # mov columns 0:rest_hi still to load after band b_last
            # rg2/rg3 only serve slots >= 2, i.e. bands >= 1: skip band-0 cols
            rest_lo23 = BAND if nb > 1 else 0

            # The four pair-0 transfers are split across GpSimd (SWDGE,
            # lowest latency) and Sync so their desc-gens run in parallel;
            # pair 2 reuses pair 0's row groups/data, so the first TWO
            # pipeline steps run off these four transfers while pair 1's
            # inputs (Scalar + Sync) land one cadence later. The abrep
            # copies' 42 x 4KB packets go LAST so they never sit ahead of
            # mov columns on the shared DMA engines.
            nc.gpsimd.dma_start(*mov_rg(0, lb0, lb1))
            nc.gpsimd.dma_start(*stat_rg(1))
            nc.gpsimd.dma_start(bapt[:], bapt_d[:])
            if rest_hi > 0:
                nc.gpsimd.dma_start(*mov_rg(0, 0, rest_hi))
                nc.gpsimd.dma_start(*mov_rg(1, 0, rest_hi))
            if rest_hi > rest_lo23 and n_lslot > 2:
                nc.gpsimd.dma_start(*mov_rg(2, rest_lo23, rest_hi))
                nc.gpsimd.dma_start(*mov_rg(3, rest_lo23, rest_hi))
            nc.gpsimd.dma_start(abrep[0:42, :], abrep_d[0:42, :])
            # Replicate the 42 abrep rows to partitions 64:106 on-device
            # (SBUF->SBUF): 172KB less HBM traffic per core, which also
            # trims cross-core HBM contention (the final barrier waits for
            # the slowest of the 8 cores' DMA streams).
            nc.gpsimd.dma_start(abrep[64:106, :], abrep[0:42, :])

            nc.sync.dma_start(*stat_rg(0))
            if n_lslot > 2:
                nc.sync.dma_start(*stat_rg(2))
                nc.sync.dma_start(*mov_rg(2, lb0, lb1))

            # Scalar (HWDGE qAct): desc-gen starts after the hoisted ACT
            # table load; mov1 first (the 4th pair-0 critical gets a
            # first-slot landing), then the rest of pair 1, then the exps.
            nc.scalar.dma_start(*mov_rg(1, lb0, lb1))
            if n_lslot > 2:
                nc.scalar.dma_start(*stat_rg(3))
                nc.scalar.dma_start(*mov_rg(3, lb0, lb1))

            # --- main loop: largest band first, 1-pair mm1 lookahead ------
            # In the first band, even pairs run before odd pairs: evens use
            # row groups 0/1 whose stat/mov land first (SWDGE), so the
            # pipeline fills while the odd pairs' transfers (HWDGE) arrive.
            seq = []
            for bi, b in enumerate(reversed(range(nb))):
                pairs = list(range(b + 1))
                if bi == 0:
                    pairs = [p for p in pairs if p % 2 == 0] + [
                        p for p in pairs if p % 2 == 1
                    ]
                for j, pair in enumerate(pairs):
                    seq.append((bi, b, pair, j == 0, j == len(pairs) - 1))

            mba = {}

            def get_mba(bi):
                if bi not in mba:
                    m = apool.tile([128, BAND], f32, tag="mba")
                    mba[bi] = m
                return mba[bi]

            gp_of = {}

            def emit_mm1(key):
                _, b, pair, _, _ = key
                gp = gpool.tile([128, 1024], f32, tag="g")
                gp_of[key] = gp
                for t in range(2):
                    s = 2 * pair + t
                    rg = _rg(s)
                    ck = s // 4
                    nc.tensor.matmul(
                        gp[:, t * BAND : (t + 1) * BAND],
                        ft_stat[32 * rg : 32 * rg + 15, ck * BLK : (ck + 1) * BLK],
                        ft_mov[32 * rg : 32 * rg + 15, b * BAND : (b + 1) * BAND],
                        start=True,
                        stop=True,
                        tile_position=(32 * rg, 0),
                    )

            emit_mm1(seq[0])
            for i, key in enumerate(seq):
                bi, b, pair, first, last = key
                if i + 1 < len(seq):
                    emit_mm1(seq[i + 1])
                gp = gp_of.pop(key)
                g_sb = gsbpool.tile([128, 1024], bf16, tag="gsb")
                nc.scalar.activation(
                    g_sb[:], gp[:], mybir.ActivationFunctionType.Exp
                )
                m_ba = get_mba(bi)
                for t in range(2):
                    s = 2 * pair + t
                    w = W2 if s < 2 * b else K_CLS  # A-side only above diag
                    col = 64 * (s % 2)
                    nc.tensor.matmul(
                        m_ba[col : col + w, :],
                        bapt[:, s * W2 : s * W2 + w],
                        g_sb[:, t * BAND : (t + 1) * BAND],
                        start=first,
                        stop=(last and t == 1),
                        tile_position=(0, col),
                        skip_group_check=True,
                    )
                if last:
                    # One STT over partitions 0:106 covers both col-group
                    # row strips (0:42 via abrep rows 0:42, 64:106 via the
                    # copy at 64:106). In-between rows read garbage PSUM /
                    # SBUF — per-partition isolated; the host masks them
                    # (and additionally masks 21:42 / 85:106 for b == 0,
                    # which has no below-diagonal tiles writing the A-side).
                    sc0 = scrpool.tile([128, BAND], bf16, tag="sc")
                    nc.vector.scalar_tensor_tensor(
                        sc0[0:106, :],
                        m_ba[0:106, :],
                        1.0,
                        abrep[0:106, b * BAND : (b + 1) * BAND],
                        mybir.AluOpType.mult,
                        mybir.AluOpType.mult,
                        accum_out=acc[0:106, b : b + 1],
                    )

            nc.sync.dma_start(acc_d[:], acc[0:106, :])

    nc.compile()
    return nc


def _host_prep(images, segmentations, ROIs, seg_label):
    """Returns (n_lslot, per-core input dicts). Core c -> image c//2, half
    c%2. Core half h owns global row blocks 2s+h of the gathered pixel set,
    s in [0, n_lslot)."""
    imgs = images[:, :, ::2, ::2].astype(np.float64)  # [N,3,64,64]
    segs = (
        segmentations.astype(np.float64)
        .reshape(N_IMG, K_CLS, H_DS, 2, H_DS, 2)
        .mean(axis=(3, 5))
    )  # [N,21,64,64]
    rois = ROIs[:, ::2, ::2].astype(np.float64)  # [N,64,64]
    lbl = seg_label[:, 0, ::2, ::2]  # [N,64,64] int32
    unlabel = lbl == IGNORE_LABEL

    seg_max = segs.max(axis=1)
    gate = rois - seg_max
    gate = np.where(unlabel, 1.0, gate)
    gate = np.maximum(gate, 0.0)  # [N,64,64]
    seg_r = segs * rois[:, None]  # [N,21,64,64]

    yy, xx = np.meshgrid(
        np.arange(H_DS, dtype=np.float64),
        np.arange(H_DS, dtype=np.float64),
        indexing="ij",
    )
    f = np.concatenate(
        [
            np.broadcast_to((xx / SIGMA_XY_EFF).reshape(1, 1, P_FULL), (N_IMG, 1, P_FULL)),
            np.broadcast_to((yy / SIGMA_XY_EFF).reshape(1, 1, P_FULL), (N_IMG, 1, P_FULL)),
            imgs.reshape(N_IMG, 3, P_FULL) / SIGMA_RGB,
        ],
        axis=1,
    )  # [N, 5, P_FULL]
    sq = (f * f).sum(axis=1)  # [N, P_FULL]
    e = np.exp(-0.5 * sq)  # [N, P_FULL]

    Bp_full = seg_r.reshape(N_IMG, K_CLS, P_FULL) * e[:, None, :]  # [N,21,P]
    Ap_full = Bp_full * gate.reshape(N_IMG, 1, P_FULL)

    # Gather ROI-active pixels (B' == A' == 0 elsewhere: exact reduction).
    act_idx = [np.flatnonzero(rois[i].ravel()) for i in range(N_IMG)]
    n_max = max(len(ix) for ix in act_idx)
    n_lslot = max(2, 2 * ((n_max + 511) // 512))  # even, P_act >= n_max
    p_act = 256 * n_lslot
    rg_slots = _rg_slots(n_lslot)

    f32 = np.float32
    in_maps = []
    for core in range(8):
        img_i = core // 2
        half = core % 2
        ix = act_idx[img_i]
        n = len(ix)

        fi = np.zeros((5, p_act), np.float64)
        fi[:, :n] = f[img_i][:, ix]
        Bp = np.zeros((K_CLS, p_act), np.float64)
        Bp[:, :n] = Bp_full[img_i][:, ix]
        Ap = np.zeros((K_CLS, p_act), np.float64)
        Ap[:, :n] = Ap_full[img_i][:, ix]

        f_32 = fi.astype(f32)
        f_hi = f_32.astype(BF16)
        f_lo = (f_32 - f_hi.astype(f32)).astype(BF16)  # [5,p_act] each

        # mov_src: [hi; hi; lo] rows, replicated x2 so the per-rg DMAs read
        # distinct DRAM rows.
        mov_15 = np.concatenate([f_hi, f_hi, f_lo], axis=0)  # [15, p_act]
        mov_src = np.concatenate([mov_15, mov_15], axis=0)  # [30, p_act]

        # stat_src: rg-major packed slot chunks. Slot s holds [hi; lo; hi]
        # of global block 2s+half; rg r's slots are contiguous columns.
        stat_src = np.zeros((15, n_lslot * BLK), BF16)
        bapt = np.zeros((128, n_lslot * W2), BF16)
        BpT = np.ascontiguousarray(Bp.T).astype(BF16)  # [p_act, 21]
        ApT = np.ascontiguousarray(Ap.T).astype(BF16)  # [p_act, 21]
        coff = 0
        for r in range(4):
            for s in rg_slots[r]:
                blk = 2 * s + half
                pix = slice(blk * BLK, (blk + 1) * BLK)
                cols = slice(coff, coff + BLK)
                stat_src[0:5, cols] = f_hi[:, pix]
                stat_src[5:10, cols] = f_lo[:, pix]
                stat_src[10:15, cols] = f_hi[:, pix]
                coff += BLK
        for s in range(n_lslot):
            blk = 2 * s + half
            pix = slice(blk * BLK, (blk + 1) * BLK)
            bapt[:, s * W2 : s * W2 + K_CLS] = BpT[pix]
            bapt[:, s * W2 + K_CLS : (s + 1) * W2] = ApT[pix]

        # abrep_src: rows 0-20 A', 21-41 B' (the device replicates them to
        # partitions 64-105 itself via an SBUF->SBUF DMA).
        abrep_src = np.zeros((42, p_act), BF16)
        abrep_src[0:K_CLS] = Ap.astype(BF16)
        abrep_src[K_CLS:W2] = Bp.astype(BF16)

        in_maps.append(
            {
                "mov_src": mov_src,
                "stat_src": stat_src,
                "bapt": bapt,
                "abrep_src": abrep_src,
            }
        )
    return n_lslot, in_maps


def _get_program(n_lslot):
    key = ("nc", n_lslot)
    if key not in _CACHE:
        _CACHE[key] = _build_program(n_lslot)
    return _CACHE[key]


def _install_profile_hook():
    """Best-effort registration of the axon NTFF profile hook so that
    trace=True works (used by test harness, not the plain kernel path)."""
    import sys
    import types

    if "antenv.axon_hooks" in sys.modules:
        return
    try:
        from trn_agent_boot.trn_boot import _ntff_profile_via_ctypes

        hook = _ntff_profile_via_ctypes("/opt/axon/libaxon_pjrt.so")
        mod = types.ModuleType("antenv.axon_hooks")
        mod.get_axon_ntff_profile_hook = lambda: hook
        sys.modules["antenv.axon_hooks"] = mod
    except Exception:
        pass


def kernel(images, segmentations, ROIs, seg_label, _trace=False, _tmpdir=None):
    from concourse import bass_utils

    n_lslot, in_maps = _host_prep(images, segmentations, ROIs, seg_label)
    nc = _get_program(n_lslot)
    if _trace:
        _install_profile_hook()
        bass_utils.upload_artifacts = lambda tmpdir: f"local:{tmpdir}"
    res = bass_utils.run_bass_kernel_spmd(
        nc, in_maps, list(range(8)), trace=_trace, tmpdir=_tmpdir
    )
    nb = n_lslot // 2
    total = 0.0
    for r in res.results:
        a = r["acc_out"].astype(np.float64)
        for b in range(nb):
            wb = W2 if b >= 1 else K_CLS
            total += a[0:wb, b].sum() + a[64 : 64 + wb, b].sum()
    loss = np.float32(-WEIGHT / N_IMG * total)
    if _trace:
        return np.array([loss], np.float32), res
    return np.array([loss], np.float32)
